# revision 2
# baseline (speedup 1.0000x reference)
"""Masked 3-layer MLP (tanh) on 8 Trainium2 NeuronCores.

Reference computation (B=2048, dims 4096->8192->8192->4096, fp32):
    h1 = tanh(x @ (W1*m1).T + b1)
    h2 = tanh(h1 @ (W2*m2).T + b2)
    out =      h2 @ (W3*m3).T + b3

Fast path (ultra-sparse masks, p~1e-4): per-core backward sparsity
closure. Core k owns out rows [k*512,(k+1)*512). Working backwards:
  S2_k = nonzero cols of m3 shard   (h2 features the shard reads)
  S1_k = nonzero cols of m2[S2_k]   (h1 features those need)
  X_k  = nonzero cols of m1[S1_k]   (x dims those need)
With p=1e-4 these are ~440/~360/~150 per core, so after padding to
multiples of 128 the whole network collapses to three tiny dense
matmuls per core with NO inter-core communication: the host packs
(W*m) submatrices (transposed, [K,F]) and a gathered x^T, and each
core independently produces its 512-row slice of out^T.

Biases are folded into the weights: one zero-padding row of x is set
to 1.0 so row |X_k| of w1t carries b1; a padding column of each hidden
layer is driven to tanh(30)=1.0 so the next layer's weight row carries
its bias. Activations then need no bias operand and layer 3's PSUM is
evicted by a plain DVE copy.

All operands and activations stay resident in SBUF ([features, batch]
orientation); output features land on PSUM partitions so tanh fuses
into the ScalarE eviction exactly as in the dense path.

Dense fallback (masks not sparse): Megatron-style column parallelism,
on-chip AllGather after layers 1 and 2 (the original implementation,
kept intact below).
"""

import os
import sys

import numpy as np

for _p in ("/opt/trn_rl_repo", os.path.expanduser("~/.axon_site/_ro/trn_rl_repo")):
    if os.path.isdir(_p) and _p not in sys.path:
        sys.path.append(_p)

B = 2048
DIMS = [4096, 8192, 8192, 4096]
NCORES = 8
P = 128
FD = 512           # matmul moving free dim == one PSUM bank of fp32
NB = B // FD       # batch blocks
ICK = 4            # K-subtiles (x128 rows) per streamed input chunk
MCK = 4            # K-subtiles per weight/mask load+mask chunk

# Compute dtype: fp16 | bf16 | fp32r | fp32
DTYPE = os.environ.get("BASS_MLP_DTYPE", "fp16")

_cache = {}
_packed_cache = {}

# Packed-path caps (SBUF residency): beyond these, use the dense path.
PK_MAX_K1 = 512
PK_MAX_F1 = 1024
PK_MAX_F2 = 1024


def _np_cdt():
    if DTYPE == "bf16":
        import ml_dtypes

        return ml_dtypes.bfloat16
    return {"fp16": np.float16, "fp32r": np.float32, "fp32": np.float32}[DTYPE]


def _mybir_cdt():
    from concourse import mybir

    return {
        "fp16": mybir.dt.float16,
        "bf16": mybir.dt.bfloat16,
        "fp32r": mybir.dt.float32r,  # rounded fp32; np side is float32
        "fp32": mybir.dt.float32,
    }[DTYPE]


# ---------------------------------------------------------------------------
# Packed (sparse-closure) path
# ---------------------------------------------------------------------------

def _pad128(n):
    return max(P, -(-n // P) * P)


def plan_packed(m1, m2, m3):
    """Per-core backward sparsity closure. Returns (dims, sets) where
    dims = (K1, F1, F2) padded shard-uniform sizes and sets[k] = (X, S1, S2)
    index arrays; or None if any core exceeds the packed-path caps."""
    m1 = np.asarray(m1)
    m2 = np.asarray(m2)
    m3 = np.asarray(m3)
    fs3 = DIMS[3] // NCORES
    sets = []
    mx = ms1 = ms2 = 0
    for k in range(NCORES):
        S2 = np.flatnonzero(m3[k * fs3:(k + 1) * fs3].any(axis=0))
        S1 = np.flatnonzero(m2[S2].any(axis=0)) if len(S2) else np.zeros(0, np.int64)
        X = np.flatnonzero(m1[S1].any(axis=0)) if len(S1) else np.zeros(0, np.int64)
        sets.append((X, S1, S2))
        mx, ms1, ms2 = max(mx, len(X)), max(ms1, len(S1)), max(ms2, len(S2))
    # +1: room for the folded-bias ones row/col.
    K1, F1, F2 = _pad128(mx + 1), _pad128(ms1 + 1), _pad128(ms2 + 1)
    if K1 > PK_MAX_K1 or F1 > PK_MAX_F1 or F2 > PK_MAX_F2:
        return None
    return (K1, F1, F2), sets


def _build_packed(K1, F1, F2):
    """Three SBUF-resident matmul layers per core, no collectives."""
    import concourse.tile as tile
    from concourse import bacc, mybir
    from concourse.bass import DynSlice

    cdt = _mybir_cdt()
    F3 = DIMS[3] // NCORES
    KO = [K1 // P, F1 // P, F2 // P]   # contraction blocks per layer
    NF = [F1 // P, F2 // P, F3 // P]   # output-feature blocks per layer

    nc = bacc.Bacc(None, target_bir_lowering=False, debug=False,
                   num_devices=NCORES)

    xT = nc.dram_tensor("xT", [K1, B], cdt, kind="ExternalInput")
    w1 = nc.dram_tensor("w1t", [K1, F1], cdt, kind="ExternalInput")
    w2 = nc.dram_tensor("w2t", [F1, F2], cdt, kind="ExternalInput")
    w3 = nc.dram_tensor("w3t", [F2, F3], cdt, kind="ExternalInput")
    out = nc.dram_tensor("out", [F3, B], mybir.dt.float32,
                         kind="ExternalOutput")

    with tile.TileContext(nc) as tc:
        with tc.tile_pool(name="wgt", bufs=1) as wpool, \
             tc.tile_pool(name="act", bufs=1) as apool, \
             tc.tile_pool(name="ev", bufs=4) as epool, \
             tc.tile_pool(name="ps", bufs=8, space="PSUM") as pspool:

            w1s = wpool.tile([P, KO[0], F1], cdt, tag="w1", name="w1s")
            w2s = wpool.tile([P, KO[1], F2], cdt, tag="w2", name="w2s")
            w3s = wpool.tile([P, KO[2], F3], cdt, tag="w3", name="w3s")
            xs = apool.tile([P, KO[0], B], cdt, tag="xs", name="xs")
            h1 = apool.tile([P, KO[1], B], cdt, tag="h1", name="h1")
            h2 = apool.tile([P, KO[2], B], cdt, tag="h2", name="h2")

            # w1 + x on the sync queue (layer 1 starts ASAP), w2/w3 behind
            # them on the gpsimd queue.
            nc.sync.dma_start(w1s[:], w1.ap().rearrange(
                "(ko p) f -> p ko f", p=P))
            xr = xT.ap().rearrange("(ko p) n -> p ko n", p=P)
            for b in range(NB):
                nc.sync.dma_start(xs[:, :, DynSlice(b * FD, FD)],
                                  xr[:, :, DynSlice(b * FD, FD)])
            nc.gpsimd.dma_start(w2s[:], w2.ap().rearrange(
                "(ko p) f -> p ko f", p=P))
            nc.gpsimd.dma_start(w3s[:], w3.ap().rearrange(
                "(ko p) f -> p ko f", p=P))

            ins = [xs, h1, h2]
            wss = [w1s, w2s, w3s]
            for li in range(3):
                for b in range(NB):
                    for f in range(NF[li]):
                        ps = pspool.tile([P, FD], mybir.dt.float32, tag="ps",
                                         name=f"ps{li}_{b}_{f}")
                        for ko in range(KO[li]):
                            nc.tensor.matmul(
                                ps[:],
                                wss[li][:, ko, DynSlice(f * P, P)],
                                ins[li][:, ko, DynSlice(b * FD, FD)],
                                start=(ko == 0), stop=(ko == KO[li] - 1))
                        if li < 2:
                            nc.scalar.activation(
                                ins[li + 1][:, f, DynSlice(b * FD, FD)],
                                ps[:], mybir.ActivationFunctionType.Tanh)
                        else:
                            ot = epool.tile([P, FD], mybir.dt.float32,
                                            tag="ot", name=f"ot{b}_{f}")
                            nc.vector.tensor_scalar_add(ot[:], ps[:], 0.0)
                            nc.sync.dma_start(
                                out.ap()[DynSlice(f * P, P),
                                         DynSlice(b * FD, FD)], ot[:])

    nc.compile()
    return nc


def get_nc_packed(dims):
    if dims not in _packed_cache:
        _packed_cache[dims] = _build_packed(*dims)
    return _packed_cache[dims]


def make_in_maps_packed(inputs, dims, sets):
    """Host-side gather/pack: masked submatrices, transposed to [K, F],
    biases folded in via a ones row of x / tanh-saturated hidden units."""
    K1, F1, F2 = dims
    F3 = DIMS[3] // NCORES
    npdt = _np_cdt()
    x = np.asarray(inputs["x"])
    Ws = [np.asarray(inputs[f"W{i}"]) for i in (1, 2, 3)]
    Ms = [np.asarray(inputs[f"m{i}"]) for i in (1, 2, 3)]
    Bs = [np.asarray(inputs[f"b{i}"]) for i in (1, 2, 3)]
    xT32 = x.T  # [4096, B] view
    in_maps = []
    for k in range(NCORES):
        X, S1, S2 = sets[k]
        nx, n1, n2 = len(X), len(S1), len(S2)
        sh = slice(k * F3, (k + 1) * F3)

        xk = np.zeros((K1, B), npdt)
        xk[:nx] = xT32[X].astype(npdt)
        xk[nx] = 1.0                      # folded-bias ones row

        w1k = np.zeros((K1, F1), np.float32)
        w1k[:nx, :n1] = (Ws[0][S1][:, X] * Ms[0][S1][:, X]).T
        w1k[nx, :n1] = Bs[0][S1]
        w1k[nx, n1] = 30.0                # h1[n1] = tanh(30) = 1

        w2k = np.zeros((F1, F2), np.float32)
        w2k[:n1, :n2] = (Ws[1][S2][:, S1] * Ms[1][S2][:, S1]).T
        w2k[n1, :n2] = Bs[1][S2]
        w2k[n1, n2] = 30.0                # h2[n2] = tanh(30) = 1

        w3k = np.zeros((F2, F3), np.float32)
        w3k[:n2] = (Ws[2][sh][:, S2] * Ms[2][sh][:, S2]).T
        w3k[n2] = Bs[2][sh]

        in_maps.append({
            "xT": xk,
            "w1t": w1k.astype(npdt),
            "w2t": w2k.astype(npdt),
            "w3t": w3k.astype(npdt),
        })
    return in_maps


# ---------------------------------------------------------------------------
# Dense fallback path (original implementation)
# ---------------------------------------------------------------------------

def _build(l1k=DIMS[0]):
    """Build + schedule the SPMD Bass program (same NEFF on all 8 cores).

    l1k: layer-1 contraction size. DIMS[0] for the dense path; a smaller
    multiple of 512 when the host packs only the K-rows that survive m1
    (per-core), padding with zeros.
    """
    import concourse.tile as tile
    from concourse import bacc, mybir
    from concourse.bass import DynSlice

    cdt = _mybir_cdt()
    esz = mybir.dt.size(cdt)

    # Per-layer output-feature shard sizes and weight-panel widths.
    FS = [DIMS[1] // NCORES, DIMS[2] // NCORES, DIMS[3] // NCORES]  # 1024,1024,512
    KS = [l1k, DIMS[1], DIMS[2]]
    if esz == 2:
        # Uniform 64KB/partition weight-panel slots so wpool can double-buffer:
        # the next panel's DMA+mask overlaps the current panel's matmuls.
        FBLK = [1024, 512, 512]
        mck, ibufs, wbufs = MCK, 6, 2
    else:
        FBLK = [1024, 512, 512]      # L2 split into two panels (SBUF)
        mck, ibufs, wbufs = 2, 4, 1

    nc = bacc.Bacc(None, target_bir_lowering=False, debug=False, num_devices=NCORES)

    xT = nc.dram_tensor("xT", [KS[0], B], cdt, kind="ExternalInput")
    wts, mts, bs = [], [], []
    for li in range(3):
        wts.append(nc.dram_tensor(f"w{li + 1}t", [KS[li], FS[li]], cdt,
                                  kind="ExternalInput"))
        mts.append(nc.dram_tensor(f"m{li + 1}t", [KS[li], FS[li]], cdt,
                                  kind="ExternalInput"))
        bs.append(nc.dram_tensor(f"b{li + 1}", [FS[li]], mybir.dt.float32,
                                 kind="ExternalInput"))
    out = nc.dram_tensor("out", [FS[2], B], mybir.dt.float32,
                         kind="ExternalOutput")

    with tile.TileContext(nc) as tc:
        with tc.tile_pool(name="wp", bufs=wbufs) as wpool, \
             tc.tile_pool(name="inp", bufs=ibufs) as ipool, \
             tc.tile_pool(name="mp", bufs=2) as mpool, \
             tc.tile_pool(name="op", bufs=6) as opool, \
             tc.tile_pool(name="bp", bufs=3) as bpool, \
             tc.tile_pool(name="ps", bufs=8, space="PSUM") as pspool, \
             tc.tile_pool(name="dram", bufs=1, space="DRAM") as dram:

            # Per-(layer, b-block) activation tensors so each AllGather covers
            # one 512-batch block and pipelines behind compute.
            h_loc = [[dram.tile([FS[li], FD], cdt, name=f"h{li + 1}_loc{b}")
                      for b in range(NB)] for li in range(2)]
            h_full = [[dram.tile([DIMS[li + 1], FD], cdt, addr_space="Shared",
                                 name=f"h{li + 1}_full{b}")
                       for b in range(NB)] for li in range(2)]

            def layer(li, tanh):
                K, F = KS[li], FS[li]
                KO = K // P
                wt_r = wts[li].ap().rearrange("(ko p) f -> p ko f", p=P)
                mt_r = mts[li].ap().rearrange("(ko p) f -> p ko f", p=P)
                if li == 0:
                    xr = xT.ap().rearrange("(ko p) n -> p ko n", p=P)
                    in_rs = [xr[:, :, DynSlice(b * FD, FD)] for b in range(NB)]
                else:
                    in_rs = [h_full[li - 1][b][:].rearrange(
                        "(ko p) n -> p ko n", p=P) for b in range(NB)]

                btile = bpool.tile([P, F // P], mybir.dt.float32, tag="bias",
                                   name=f"bias{li}")
                nc.sync.dma_start(btile[:], bs[li].ap().rearrange(
                    "(o p) -> p o", p=P))

                fblk = FBLK[li]
                for f0 in range(0, F, fblk):
                    # --- load + mask one weight panel [P, KO, fblk] ---
                    wp = wpool.tile([P, KO, fblk], cdt, tag="wpanel",
                                    name=f"wp{li}_{f0}")
                    # weight/mask loads go on gpsimd/vector DMA queues so the
                    # input-strip stream on the sync queue is never stuck
                    # behind a 16MB panel load
                    for c0 in range(0, KO, mck):
                        csl = slice(c0, c0 + mck)
                        fsl = DynSlice(f0, fblk)
                        nc.gpsimd.dma_start(wp[:, csl, :], wt_r[:, csl, fsl])
                        mtile = mpool.tile([P, mck, fblk], cdt, tag="mchunk",
                                           name=f"m{li}_{f0}_{c0}")
                        nc.gpsimd.dma_start(mtile[:], mt_r[:, csl, fsl])
                        nc.vector.tensor_tensor(wp[:, csl, :], wp[:, csl, :],
                                                mtile[:], mybir.AluOpType.mult)

                    nf = fblk // P
                    for b in range(NB):
                        psums = [pspool.tile([P, FD], mybir.dt.float32,
                                             tag="ps", name=f"ps{li}_{f0}_{b}_{f}")
                                 for f in range(nf)]
                        for c0 in range(0, KO, ICK):
                            it = ipool.tile([P, ICK, FD], cdt, tag="instrip",
                                            name=f"in{li}_{f0}_{b}_{c0}")
                            nc.sync.dma_start(
                                it[:], in_rs[b][:, slice(c0, c0 + ICK), :])
                            for f in range(nf):
                                for ks in range(ICK):
                                    ko = c0 + ks
                                    nc.tensor.matmul(
                                        psums[f][:],
                                        wp[:, ko, DynSlice(f * P, P)],
                                        it[:, ks, :],
                                        start=(ko == 0), stop=(ko == KO - 1))
                        for f in range(nf):
                            fg = f0 + f * P   # feature row offset in shard
                            odt = cdt if li < 2 else mybir.dt.float32
                            ot = opool.tile([P, FD], odt, tag="prod",
                                            name=f"o{li}_{f0}_{b}_{f}")
                            func = (mybir.ActivationFunctionType.Tanh if tanh
                                    else mybir.ActivationFunctionType.Identity)
                            nc.scalar.activation(
                                ot[:], psums[f][:], func,
                                bias=btile[:, DynSlice((f0 // P) + f, 1)])
                            if li < 2:
                                nc.sync.dma_start(
                                    h_loc[li][b][DynSlice(fg, P), :], ot[:])
                            else:
                                nc.sync.dma_start(
                                    out.ap()[DynSlice(fg, P),
                                             DynSlice(b * FD, FD)], ot[:])
                        # fire this b-block's AllGather as soon as the last
                        # panel has written it
                        if li < 2 and f0 == F - fblk:
                            nc.gpsimd.collective_compute(
                                "AllGather",
                                mybir.AluOpType.bypass,
                                replica_groups=[list(range(NCORES))],
                                ins=[h_loc[li][b].opt()],
                                outs=[h_full[li][b].opt()],
                            )

            layer(0, tanh=True)
            layer(1, tanh=True)
            layer(2, tanh=False)

    nc.compile()
    return nc


PACK_K = 512   # packed layer-1 contraction size (sparse-mask fast path)


def get_nc(l1k=DIMS[0]):
    if l1k not in _cache:
        _cache[l1k] = _build(l1k)
    return _cache[l1k]


def plan_l1k(m1):
    """If m1 is sparse enough that every core's shard of (W1*m1).T touches at
    most PACK_K input dims, return (PACK_K, per-core used-row indices); else
    the dense plan."""
    m1 = np.asarray(m1)
    fs = DIMS[1] // NCORES
    idxs = []
    for k in range(NCORES):
        idx = np.flatnonzero(m1[k * fs:(k + 1) * fs].any(axis=0))
        if len(idx) > PACK_K:
            return DIMS[0], None
        idxs.append(idx)
    return PACK_K, idxs


def make_in_maps(x, W1, b1, m1, W2, b2, m2, W3, b3, m3, idxs=None):
    """Host-side sharding: transpose to [K, F] layouts, cast, slice shards.
    With idxs, layer-1 operands are gathered to the PACK_K used K-rows."""
    x, W1, b1, m1, W2, b2, m2, W3, b3, m3 = (
        np.asarray(a) for a in (x, W1, b1, m1, W2, b2, m2, W3, b3, m3))
    npdt = _np_cdt()
    xT = np.ascontiguousarray(x.T).astype(npdt, copy=False)
    Ws = [W1, W2, W3]
    Ms = [m1, m2, m3]
    Bs = [b1, b2, b3]
    in_maps = []
    for k in range(NCORES):
        m = {}
        for li in range(3):
            F = DIMS[li + 1]
            fs = F // NCORES
            sl = slice(k * fs, (k + 1) * fs)
            wt = Ws[li][sl].T
            mt = Ms[li][sl].T
            if li == 0:
                if idxs is None:
                    m["xT"] = xT
                else:
                    idx = idxs[k]
                    xk = np.zeros((PACK_K, B), npdt)
                    xk[:len(idx)] = xT[idx]
                    m["xT"] = xk
                    wk = np.zeros((PACK_K, fs), npdt)
                    wk[:len(idx)] = wt[idx].astype(npdt)
                    mk = np.zeros((PACK_K, fs), npdt)
                    mk[:len(idx)] = mt[idx].astype(npdt)
                    m["w1t"], m["m1t"] = wk, mk
            if f"w{li + 1}t" not in m:
                m[f"w{li + 1}t"] = np.ascontiguousarray(wt).astype(
                    npdt, copy=False)
                m[f"m{li + 1}t"] = np.ascontiguousarray(mt).astype(npdt)
            m[f"b{li + 1}"] = np.ascontiguousarray(Bs[li][sl]).astype(
                np.float32, copy=False)
        in_maps.append(m)
    return in_maps


# ---------------------------------------------------------------------------
# Entry
# ---------------------------------------------------------------------------

def prepare(inputs):
    """Plan + build + pack. Returns (nc, in_maps)."""
    plan = plan_packed(inputs["m1"], inputs["m2"], inputs["m3"])
    if plan is not None:
        dims, sets = plan
        nc = get_nc_packed(dims)
        return nc, make_in_maps_packed(inputs, dims, sets)
    l1k, idxs = plan_l1k(inputs["m1"])
    nc = get_nc(l1k)
    return nc, make_in_maps(**inputs, idxs=idxs)


def kernel(x, W1, b1, m1, W2, b2, m2, W3, b3, m3):
    from concourse.bass_utils import run_bass_kernel_spmd

    inputs = dict(x=x, W1=W1, b1=b1, m1=m1, W2=W2, b2=b2, m2=m2,
                  W3=W3, b3=b3, m3=m3)
    nc, in_maps = prepare(inputs)
    res = run_bass_kernel_spmd(nc, in_maps, core_ids=list(range(NCORES)))
    outT = np.concatenate([res.results[k]["out"] for k in range(NCORES)], axis=0)
    return np.ascontiguousarray(outT.T)


# revision 24
# speedup vs baseline: 1.0124x; 1.0124x over previous
"""Masked 3-layer MLP (tanh) on 8 Trainium2 NeuronCores.

Reference computation (B=2048, dims 4096->8192->8192->4096, fp32):
    h1 = tanh(x @ (W1*m1).T + b1)
    h2 = tanh(h1 @ (W2*m2).T + b2)
    out =      h2 @ (W3*m3).T + b3

Fast path (ultra-sparse masks, p~1e-4): per-core backward sparsity
closure. Core k owns out rows [k*512,(k+1)*512). Working backwards:
  S2_k = nonzero cols of m3 shard   (h2 features the shard reads)
  S1_k = nonzero cols of m2[S2_k]   (h1 features those need)
  X_k  = nonzero cols of m1[S1_k]   (x dims those need)
With p=1e-4 these are ~440/~360/~150 per core, so after padding to
multiples of 128 the whole network collapses to three tiny dense
matmuls per core with NO inter-core communication: the host packs
(W*m) submatrices (transposed, [K,F]) and a gathered x^T, and each
core independently produces its 512-row slice of out^T.

Biases are folded into the weights: one zero-padding row of x is set
to 1.0 so row |X_k| of w1t carries b1; a padding column of each hidden
layer is driven to tanh(30)=1.0 so the next layer's weight row carries
its bias. Activations then need no bias operand and layer 3's PSUM is
evicted by a plain DVE copy.

All operands and activations stay resident in SBUF ([features, batch]
orientation); output features land on PSUM partitions so tanh fuses
into the ScalarE eviction exactly as in the dense path.

Dense fallback (masks not sparse): Megatron-style column parallelism,
on-chip AllGather after layers 1 and 2 (the original implementation,
kept intact below).
"""

import os
import sys

import numpy as np

for _p in ("/opt/trn_rl_repo", os.path.expanduser("~/.axon_site/_ro/trn_rl_repo")):
    if os.path.isdir(_p) and _p not in sys.path:
        sys.path.append(_p)

B = 2048
DIMS = [4096, 8192, 8192, 4096]
NCORES = 8
P = 128
FD = 512           # matmul moving free dim == one PSUM bank of fp32
NB = B // FD       # batch blocks
ICK = 4            # K-subtiles (x128 rows) per streamed input chunk
MCK = 4            # K-subtiles per weight/mask load+mask chunk

# Compute dtype: fp16 | bf16 | fp32r | fp32
DTYPE = os.environ.get("BASS_MLP_DTYPE", "fp16")

_cache = {}
_packed_cache = {}

# Packed-path caps (SBUF residency): beyond these, use the dense path.
PK_MAX_K1 = 512
PK_MAX_F1 = 1024
PK_MAX_F2 = 1024


def _np_cdt():
    if DTYPE == "bf16":
        import ml_dtypes

        return ml_dtypes.bfloat16
    return {"fp16": np.float16, "fp32r": np.float32, "fp32": np.float32}[DTYPE]


def _mybir_cdt():
    from concourse import mybir

    return {
        "fp16": mybir.dt.float16,
        "bf16": mybir.dt.bfloat16,
        "fp32r": mybir.dt.float32r,  # rounded fp32; np side is float32
        "fp32": mybir.dt.float32,
    }[DTYPE]


# ---------------------------------------------------------------------------
# Packed (sparse-closure) path
# ---------------------------------------------------------------------------

def _pad128(n):
    return max(P, -(-n // P) * P)


def plan_packed(m1, m2, m3):
    """Per-core backward sparsity closure over NON-CONSTANT features only.

    An h1 feature is constant (= tanh(b1)) when its m1 row is empty; an h2
    feature is constant when its m2 row touches only constant h1 features.
    Constant features' contributions are folded into downstream biases on
    the host (make_in_maps_packed), so the device network only carries
    features with a live path from x.

    Returns (dims, sets, nonc1, nonc2) where dims = (K1, F1, F2) padded
    shard-uniform sizes and sets[k] = (X, S1, S2) index arrays; or None if
    any core exceeds the packed-path caps."""
    m1 = np.asarray(m1)
    m2 = np.asarray(m2)
    m3 = np.asarray(m3)
    nonc1 = m1.any(axis=1)                      # h1 features with live input
    nonc2 = m2[:, nonc1].any(axis=1)            # h2 features with live input
    fs3 = DIMS[3] // NCORES
    sets = []
    mx = ms1 = ms2 = 0
    for k in range(NCORES):
        sh3 = m3[k * fs3:(k + 1) * fs3]
        S2 = np.flatnonzero(sh3.any(axis=0) & nonc2)
        S1 = (np.flatnonzero(m2[S2].any(axis=0) & nonc1)
              if len(S2) else np.zeros(0, np.int64))
        X = (np.flatnonzero(m1[S1].any(axis=0))
             if len(S1) else np.zeros(0, np.int64))
        sets.append((X, S1, S2))
        mx, ms1, ms2 = max(mx, len(X)), max(ms1, len(S1)), max(ms2, len(S2))
    # +1: room for the folded-bias ones row/col.
    K1, F1, F2 = _pad128(mx + 1), _pad128(ms1 + 1), _pad128(ms2 + 1)
    if K1 > PK_MAX_K1 or F1 > PK_MAX_F1 or F2 > PK_MAX_F2:
        return None
    return (K1, F1, F2), sets, nonc1, nonc2


def _build_packed(K1, F1, F2):
    """Three SBUF-resident matmul layers per core, no collectives."""
    import concourse.tile as tile
    from concourse import bacc, mybir
    from concourse.bass import DynSlice

    cdt = _mybir_cdt()
    F3 = DIMS[3] // NCORES
    KO = [K1 // P, F1 // P, F2 // P]   # contraction blocks per layer
    NF = [F1 // P, F2 // P, F3 // P]   # output-feature blocks per layer

    nc = bacc.Bacc(None, target_bir_lowering=False, debug=False,
                   num_devices=NCORES)

    xT = nc.dram_tensor("xT", [K1, B], cdt, kind="ExternalInput")
    w1 = nc.dram_tensor("w1t", [K1, F1], cdt, kind="ExternalInput")
    w2 = nc.dram_tensor("w2t", [F1, F2], cdt, kind="ExternalInput")
    w3 = nc.dram_tensor("w3t", [F2, F3], cdt, kind="ExternalInput")
    out = nc.dram_tensor("out", [F3, B], cdt, kind="ExternalOutput")

    FD2 = 2 * FD          # batch-pair granularity: 1024 cols, 2 PSUM banks
    NBP = B // FD2        # 2 batch-pairs

    with tile.TileContext(nc) as tc:
        with tc.tile_pool(name="wgt", bufs=1) as wpool, \
             tc.tile_pool(name="act", bufs=1) as apool, \
             tc.tile_pool(name="ev", bufs=16) as epool, \
             tc.tile_pool(name="ps", bufs=1, space="PSUM") as pspool:

            w1s = wpool.tile([P, KO[0], F1], cdt, tag="w1", name="w1s")
            w2s = wpool.tile([P, KO[1], F2], cdt, tag="w2", name="w2s")
            w3s = wpool.tile([P, KO[2], F3], cdt, tag="w3", name="w3s")
            xs = apool.tile([P, KO[0], B], cdt, tag="xs", name="xs")
            h1 = apool.tile([P, KO[1], B], cdt, tag="h1", name="h1")
            h2 = apool.tile([P, KO[2], B], cdt, tag="h2", name="h2")

            # First-needed inputs leave on separate queues in parallel:
            # x block 0 heads the SP queue while w1 heads the Pool queue
            # (both gate the first matmul); ACT's queue only carries x b2
            # behind its auto-inserted tanh-table load.
            xr = xT.ap().rearrange("(ko p) n -> p ko n", p=P)
            xq = [nc.sync, nc.sync, nc.scalar, nc.sync]
            nc.gpsimd.dma_start(w1s[:], w1.ap().rearrange(
                "(ko p) f -> p ko f", p=P))
            for b in range(NB):
                xq[b].dma_start(xs[:, :, DynSlice(b * FD, FD)],
                                xr[:, :, DynSlice(b * FD, FD)])
            nc.gpsimd.dma_start(w2s[:], w2.ap().rearrange(
                "(ko p) f -> p ko f", p=P))
            nc.gpsimd.dma_start(w3s[:], w3.ap().rearrange(
                "(ko p) f -> p ko f", p=P))

            ins = [xs, h1, h2]
            wss = [w1s, w2s, w3s]

            # Software-pipelined (layer, batch-block) order: each stage's
            # tanh is ready just-in-time for its consumer and L3 blocks
            # start as early as their L2 tanh allows, so the PSUM drain
            # (the structural bottleneck: only DVE and ACT may read PSUM
            # on TRN2) spreads over most of the kernel.
            ORDER = [(0, 0), (0, 1), (1, 0), (0, 2), (1, 1), (2, 0),
                     (0, 3), (1, 2), (2, 1), (1, 3), (2, 2), (2, 3)]
            # Greedy drain scheduler against the v1 cost-model constants:
            # ACT joins the evict pool only for stages emitted after the
            # last tanh (its queue is in-order — an earlier evict would
            # delay the tanh chain and stall PE behind it). Pool cannot
            # touch PSUM, so it only carries stores.
            SEM = 100.0
            MM, TANH = 213.0, 612.0
            EVC = {"dve": 658.0, "act": 612.0}
            DMC = 500.0
            ENG = {"dve": nc.vector, "act": nc.scalar,
                   "pool": nc.gpsimd, "sp": nc.sync}
            last_tanh_pos = max(i for i, (li, _) in enumerate(ORDER)
                                if li < 2)
            pe_t = 2480.0 + 2 * 427.0 - 2 * MM   # first two matmuls at mid
            avail = {"dve": 200.0, "pool": 1600.0, "act": 2300.0,
                     "sp": 2600.0}
            for pos, (li, b) in enumerate(ORDER):
                for f in range(NF[li]):
                    ps = pspool.tile([P, FD], mybir.dt.float32, tag="ps",
                                     bufs=8, name=f"ps{li}_{b}_{f}")
                    for ko in range(KO[li]):
                        nc.tensor.matmul(
                            ps[:],
                            wss[li][:, ko, DynSlice(f * P, P)],
                            ins[li][:, ko, DynSlice(b * FD, FD)],
                            start=(ko == 0), stop=(ko == KO[li] - 1))
                        pe_t += MM
                    if li < 2:
                        nc.scalar.activation(
                            ins[li + 1][:, f, DynSlice(b * FD, FD)],
                            ps[:], mybir.ActivationFunctionType.Tanh)
                        avail["act"] = max(avail["act"], pe_t + SEM) + TANH
                        continue
                    cands = ("dve", "act") if pos > last_tanh_pos \
                        else ("dve",)
                    evn = min(cands,
                              key=lambda e: max(avail[e], pe_t + SEM)
                              + EVC[e])
                    ev_end = max(avail[evn], pe_t + SEM) + EVC[evn]
                    avail[evn] = ev_end
                    ot = epool.tile([P, FD], cdt, tag="ot", bufs=12,
                                    name=f"ot{f}_{b}")
                    if evn == "act":
                        nc.scalar.activation(
                            ot[:], ps[:], mybir.ActivationFunctionType.Copy)
                    else:
                        ENG[evn].tensor_scalar_add(ot[:], ps[:], 0.0)
                    dqn = min(("sp", "pool"),
                              key=lambda q: max(avail[q], ev_end + SEM)
                              + DMC)
                    avail[dqn] = max(avail[dqn], ev_end + SEM) + DMC
                    ENG[dqn].dma_start(
                        out.ap()[DynSlice(f * P, P),
                                 DynSlice(b * FD, FD)], ot[:])

    nc.compile()
    return nc


def get_nc_packed(dims):
    if dims not in _packed_cache:
        _packed_cache[dims] = _build_packed(*dims)
    return _packed_cache[dims]


def make_in_maps_packed(inputs, dims, sets, nonc1, nonc2):
    """Host-side gather/pack: masked submatrices, transposed to [K, F].

    Constant-feature folding: h1 features with empty m1 rows are exactly
    tanh(b1); their layer-2 contribution is folded into b2' per h2 feature.
    h2 features whose m2 rows touch only constant h1 features are exactly
    tanh(b2'); their layer-3 contribution is folded into b3'. Remaining
    (folded) biases ride in via a ones row of x / tanh-saturated hidden
    padding units, so the device kernel is three pure matmuls + tanh."""
    K1, F1, F2 = dims
    F3 = DIMS[3] // NCORES
    npdt = _np_cdt()
    x = np.asarray(inputs["x"])
    Ws = [np.asarray(inputs[f"W{i}"]) for i in (1, 2, 3)]
    Ms = [np.asarray(inputs[f"m{i}"]) for i in (1, 2, 3)]
    Bs = [np.asarray(inputs[f"b{i}"]).astype(np.float32) for i in (1, 2, 3)]

    # Fold constant-feature contributions into downstream biases.
    t1 = np.tanh(Bs[0])                     # h1 value where m1 row empty
    c1 = ~nonc1
    r2, cc2 = np.nonzero(Ms[1])
    sel = c1[cc2]
    b2p = Bs[1].copy()
    np.add.at(b2p, r2[sel], Ws[1][r2[sel], cc2[sel]].astype(np.float32)
              * t1[cc2[sel]])
    h2c = np.tanh(b2p)                      # h2 value for constant features
    c2 = ~nonc2
    r3, cc3 = np.nonzero(Ms[2])
    sel3 = c2[cc3]
    b3p = Bs[2].copy()
    np.add.at(b3p, r3[sel3], Ws[2][r3[sel3], cc3[sel3]].astype(np.float32)
              * h2c[cc3[sel3]])

    xT32 = x.T  # [4096, B] view
    in_maps = []
    for k in range(NCORES):
        X, S1, S2 = sets[k]
        nx, n1, n2 = len(X), len(S1), len(S2)
        sh = slice(k * F3, (k + 1) * F3)

        xk = np.zeros((K1, B), npdt)
        xk[:nx] = xT32[X].astype(npdt)
        xk[nx] = 1.0                      # folded-bias ones row

        w1k = np.zeros((K1, F1), np.float32)
        w1k[:nx, :n1] = (Ws[0][S1][:, X] * Ms[0][S1][:, X]).T
        w1k[nx, :n1] = Bs[0][S1]
        w1k[nx, n1] = 30.0                # h1[n1] = tanh(30) = 1

        w2k = np.zeros((F1, F2), np.float32)
        w2k[:n1, :n2] = (Ws[1][S2][:, S1] * Ms[1][S2][:, S1]).T
        w2k[n1, :n2] = b2p[S2]
        w2k[n1, n2] = 30.0                # h2[n2] = tanh(30) = 1

        w3k = np.zeros((F2, F3), np.float32)
        w3k[:n2] = (Ws[2][sh][:, S2] * Ms[2][sh][:, S2]).T
        w3k[n2] = b3p[sh]

        in_maps.append({
            "xT": xk,
            "w1t": w1k.astype(npdt),
            "w2t": w2k.astype(npdt),
            "w3t": w3k.astype(npdt),
        })
    return in_maps


# ---------------------------------------------------------------------------
# Dense fallback path (original implementation)
# ---------------------------------------------------------------------------

def _build(l1k=DIMS[0]):
    """Build + schedule the SPMD Bass program (same NEFF on all 8 cores).

    l1k: layer-1 contraction size. DIMS[0] for the dense path; a smaller
    multiple of 512 when the host packs only the K-rows that survive m1
    (per-core), padding with zeros.
    """
    import concourse.tile as tile
    from concourse import bacc, mybir
    from concourse.bass import DynSlice

    cdt = _mybir_cdt()
    esz = mybir.dt.size(cdt)

    # Per-layer output-feature shard sizes and weight-panel widths.
    FS = [DIMS[1] // NCORES, DIMS[2] // NCORES, DIMS[3] // NCORES]  # 1024,1024,512
    KS = [l1k, DIMS[1], DIMS[2]]
    if esz == 2:
        # Uniform 64KB/partition weight-panel slots so wpool can double-buffer:
        # the next panel's DMA+mask overlaps the current panel's matmuls.
        FBLK = [1024, 512, 512]
        mck, ibufs, wbufs = MCK, 6, 2
    else:
        FBLK = [1024, 512, 512]      # L2 split into two panels (SBUF)
        mck, ibufs, wbufs = 2, 4, 1

    nc = bacc.Bacc(None, target_bir_lowering=False, debug=False, num_devices=NCORES)

    xT = nc.dram_tensor("xT", [KS[0], B], cdt, kind="ExternalInput")
    wts, mts, bs = [], [], []
    for li in range(3):
        wts.append(nc.dram_tensor(f"w{li + 1}t", [KS[li], FS[li]], cdt,
                                  kind="ExternalInput"))
        mts.append(nc.dram_tensor(f"m{li + 1}t", [KS[li], FS[li]], cdt,
                                  kind="ExternalInput"))
        bs.append(nc.dram_tensor(f"b{li + 1}", [FS[li]], mybir.dt.float32,
                                 kind="ExternalInput"))
    out = nc.dram_tensor("out", [FS[2], B], mybir.dt.float32,
                         kind="ExternalOutput")

    with tile.TileContext(nc) as tc:
        with tc.tile_pool(name="wp", bufs=wbufs) as wpool, \
             tc.tile_pool(name="inp", bufs=ibufs) as ipool, \
             tc.tile_pool(name="mp", bufs=2) as mpool, \
             tc.tile_pool(name="op", bufs=6) as opool, \
             tc.tile_pool(name="bp", bufs=3) as bpool, \
             tc.tile_pool(name="ps", bufs=8, space="PSUM") as pspool, \
             tc.tile_pool(name="dram", bufs=1, space="DRAM") as dram:

            # Per-(layer, b-block) activation tensors so each AllGather covers
            # one 512-batch block and pipelines behind compute.
            h_loc = [[dram.tile([FS[li], FD], cdt, name=f"h{li + 1}_loc{b}")
                      for b in range(NB)] for li in range(2)]
            h_full = [[dram.tile([DIMS[li + 1], FD], cdt, addr_space="Shared",
                                 name=f"h{li + 1}_full{b}")
                       for b in range(NB)] for li in range(2)]

            def layer(li, tanh):
                K, F = KS[li], FS[li]
                KO = K // P
                wt_r = wts[li].ap().rearrange("(ko p) f -> p ko f", p=P)
                mt_r = mts[li].ap().rearrange("(ko p) f -> p ko f", p=P)
                if li == 0:
                    xr = xT.ap().rearrange("(ko p) n -> p ko n", p=P)
                    in_rs = [xr[:, :, DynSlice(b * FD, FD)] for b in range(NB)]
                else:
                    in_rs = [h_full[li - 1][b][:].rearrange(
                        "(ko p) n -> p ko n", p=P) for b in range(NB)]

                btile = bpool.tile([P, F // P], mybir.dt.float32, tag="bias",
                                   name=f"bias{li}")
                nc.sync.dma_start(btile[:], bs[li].ap().rearrange(
                    "(o p) -> p o", p=P))

                fblk = FBLK[li]
                for f0 in range(0, F, fblk):
                    # --- load + mask one weight panel [P, KO, fblk] ---
                    wp = wpool.tile([P, KO, fblk], cdt, tag="wpanel",
                                    name=f"wp{li}_{f0}")
                    # weight/mask loads go on gpsimd/vector DMA queues so the
                    # input-strip stream on the sync queue is never stuck
                    # behind a 16MB panel load
                    for c0 in range(0, KO, mck):
                        csl = slice(c0, c0 + mck)
                        fsl = DynSlice(f0, fblk)
                        nc.gpsimd.dma_start(wp[:, csl, :], wt_r[:, csl, fsl])
                        mtile = mpool.tile([P, mck, fblk], cdt, tag="mchunk",
                                           name=f"m{li}_{f0}_{c0}")
                        nc.gpsimd.dma_start(mtile[:], mt_r[:, csl, fsl])
                        nc.vector.tensor_tensor(wp[:, csl, :], wp[:, csl, :],
                                                mtile[:], mybir.AluOpType.mult)

                    nf = fblk // P
                    for b in range(NB):
                        psums = [pspool.tile([P, FD], mybir.dt.float32,
                                             tag="ps", name=f"ps{li}_{f0}_{b}_{f}")
                                 for f in range(nf)]
                        for c0 in range(0, KO, ICK):
                            it = ipool.tile([P, ICK, FD], cdt, tag="instrip",
                                            name=f"in{li}_{f0}_{b}_{c0}")
                            nc.sync.dma_start(
                                it[:], in_rs[b][:, slice(c0, c0 + ICK), :])
                            for f in range(nf):
                                for ks in range(ICK):
                                    ko = c0 + ks
                                    nc.tensor.matmul(
                                        psums[f][:],
                                        wp[:, ko, DynSlice(f * P, P)],
                                        it[:, ks, :],
                                        start=(ko == 0), stop=(ko == KO - 1))
                        for f in range(nf):
                            fg = f0 + f * P   # feature row offset in shard
                            odt = cdt if li < 2 else mybir.dt.float32
                            ot = opool.tile([P, FD], odt, tag="prod",
                                            name=f"o{li}_{f0}_{b}_{f}")
                            func = (mybir.ActivationFunctionType.Tanh if tanh
                                    else mybir.ActivationFunctionType.Identity)
                            nc.scalar.activation(
                                ot[:], psums[f][:], func,
                                bias=btile[:, DynSlice((f0 // P) + f, 1)])
                            if li < 2:
                                nc.sync.dma_start(
                                    h_loc[li][b][DynSlice(fg, P), :], ot[:])
                            else:
                                nc.sync.dma_start(
                                    out.ap()[DynSlice(fg, P),
                                             DynSlice(b * FD, FD)], ot[:])
                        # fire this b-block's AllGather as soon as the last
                        # panel has written it
                        if li < 2 and f0 == F - fblk:
                            nc.gpsimd.collective_compute(
                                "AllGather",
                                mybir.AluOpType.bypass,
                                replica_groups=[list(range(NCORES))],
                                ins=[h_loc[li][b].opt()],
                                outs=[h_full[li][b].opt()],
                            )

            layer(0, tanh=True)
            layer(1, tanh=True)
            layer(2, tanh=False)

    nc.compile()
    return nc


PACK_K = 512   # packed layer-1 contraction size (sparse-mask fast path)


def get_nc(l1k=DIMS[0]):
    if l1k not in _cache:
        _cache[l1k] = _build(l1k)
    return _cache[l1k]


def plan_l1k(m1):
    """If m1 is sparse enough that every core's shard of (W1*m1).T touches at
    most PACK_K input dims, return (PACK_K, per-core used-row indices); else
    the dense plan."""
    m1 = np.asarray(m1)
    fs = DIMS[1] // NCORES
    idxs = []
    for k in range(NCORES):
        idx = np.flatnonzero(m1[k * fs:(k + 1) * fs].any(axis=0))
        if len(idx) > PACK_K:
            return DIMS[0], None
        idxs.append(idx)
    return PACK_K, idxs


def make_in_maps(x, W1, b1, m1, W2, b2, m2, W3, b3, m3, idxs=None):
    """Host-side sharding: transpose to [K, F] layouts, cast, slice shards.
    With idxs, layer-1 operands are gathered to the PACK_K used K-rows."""
    x, W1, b1, m1, W2, b2, m2, W3, b3, m3 = (
        np.asarray(a) for a in (x, W1, b1, m1, W2, b2, m2, W3, b3, m3))
    npdt = _np_cdt()
    xT = np.ascontiguousarray(x.T).astype(npdt, copy=False)
    Ws = [W1, W2, W3]
    Ms = [m1, m2, m3]
    Bs = [b1, b2, b3]
    in_maps = []
    for k in range(NCORES):
        m = {}
        for li in range(3):
            F = DIMS[li + 1]
            fs = F // NCORES
            sl = slice(k * fs, (k + 1) * fs)
            wt = Ws[li][sl].T
            mt = Ms[li][sl].T
            if li == 0:
                if idxs is None:
                    m["xT"] = xT
                else:
                    idx = idxs[k]
                    xk = np.zeros((PACK_K, B), npdt)
                    xk[:len(idx)] = xT[idx]
                    m["xT"] = xk
                    wk = np.zeros((PACK_K, fs), npdt)
                    wk[:len(idx)] = wt[idx].astype(npdt)
                    mk = np.zeros((PACK_K, fs), npdt)
                    mk[:len(idx)] = mt[idx].astype(npdt)
                    m["w1t"], m["m1t"] = wk, mk
            if f"w{li + 1}t" not in m:
                m[f"w{li + 1}t"] = np.ascontiguousarray(wt).astype(
                    npdt, copy=False)
                m[f"m{li + 1}t"] = np.ascontiguousarray(mt).astype(npdt)
            m[f"b{li + 1}"] = np.ascontiguousarray(Bs[li][sl]).astype(
                np.float32, copy=False)
        in_maps.append(m)
    return in_maps


# ---------------------------------------------------------------------------
# Entry
# ---------------------------------------------------------------------------

def prepare(inputs):
    """Plan + build + pack. Returns (nc, in_maps)."""
    plan = plan_packed(inputs["m1"], inputs["m2"], inputs["m3"])
    if plan is not None:
        dims, sets, nonc1, nonc2 = plan
        nc = get_nc_packed(dims)
        return nc, make_in_maps_packed(inputs, dims, sets, nonc1, nonc2)
    l1k, idxs = plan_l1k(inputs["m1"])
    nc = get_nc(l1k)
    return nc, make_in_maps(**inputs, idxs=idxs)


def kernel(x, W1, b1, m1, W2, b2, m2, W3, b3, m3):
    from concourse.bass_utils import run_bass_kernel_spmd

    inputs = dict(x=x, W1=W1, b1=b1, m1=m1, W2=W2, b2=b2, m2=m2,
                  W3=W3, b3=b3, m3=m3)
    nc, in_maps = prepare(inputs)
    res = run_bass_kernel_spmd(nc, in_maps, core_ids=list(range(NCORES)))
    outT = np.concatenate([res.results[k]["out"] for k in range(NCORES)], axis=0)
    return np.ascontiguousarray(outT.T, dtype=np.float32)
